# revision 8
# baseline (speedup 1.0000x reference)
"""Trainium2 Bass kernel: teacher-forced LSTM decoder + packed vocab projection.

Model (B=128, T=20, E=H=512, V=32000):
  x = [features, embed(captions[:, :T-1])]            # [B, T, E]
  (h, c) LSTM-scan over T steps (PyTorch gate order i,f,g,o)
  logits = hs @ lin_w.T + lin_b                       # [T, B, V]
  out = logits packed time-major, keeping rows with length > t  # [sum(len), V]

Strategy (8 NeuronCores, vocab-parallel):
  - Core s owns lin_w columns [s*4000, (s+1)*4000); the LSTM recurrence is
    replicated on every core.
  - x-part gates are HOISTED out of the recurrence: XG[L, 4H] = X @ w_ih.T is
    computed in one chunk-batched bf16 pass over the packed rows (13 chunks of
    128 instead of 20 per-step groups -> fewer weight-column streams), and
    doubles as the PE p-state warmup.
  - Per step t, the 4 gate PSUM banks are PREFILLED with XG rows (one scalar
    copy), then only the h-part accumulates via matmul (start=False):
      gates[n,4H] = prefill(XG) + h_{t-1} @ w_hh.T     (f32r, 4K x 4 banks)
    Gate columns are host-permuted to [i,f,o,g] so one fused sigmoid covers
    banks 0..2 and one tanh covers bank 3.
  - c update in fp32 on DVE, h transposed back to [H, n] via PE transpose,
    stash into a bf16 [H, L] tile feeding the projection.
  - Projection: per (row-chunk, 500-wide vocab slice), bf16 matmul vs resident
    bf16 linT; items are interleaved between recurrence steps as PE filler.
  - Host gathers the 8 core outputs and concatenates along vocab.
"""

import math

import numpy as np

import concourse.bacc as bacc
import concourse.bass as bass
import concourse.mybir as mybir
import concourse.tile as tile
from concourse.bass_utils import run_bass_kernel_spmd

B, T, E, H, V = 128, 20, 512, 512, 32000
NCORES = 8
VS = V // NCORES      # per-core vocab shard (4000)
NV = 8                # vocab sub-chunks per core
VC = VS // NV         # 500 columns per projection matmul (>=256 keeps rate 1c/r)
KE = E // 128         # 4 contraction chunks over E
KH = H // 128         # 4 contraction chunks over H
P = 128

F32 = mybir.dt.float32
F32R = mybir.dt.float32r
BF16 = mybir.dt.bfloat16
AF = mybir.ActivationFunctionType

# gate blocks reordered host-side from PyTorch [i,f,g,o] to [i,f,o,g] so the
# three sigmoids are contiguous PSUM banks 0..2 and tanh is bank 3
GPERM = (0, 1, 3, 2)

# PE filler sizing: projection items emitted between a step's h-matmuls and
# its transpose (F1) and after the transpose (F2)
F1 = 3
F2 = 1


def build_program(n_ts, use_bias, use_linb, debug_dump=False):
    """Build the single-core Bass/Tile program (same program on all 8 cores).

    n_ts: per-step active-row counts (descending, all > 0), len(n_ts) <= T.
    """
    L = int(sum(n_ts))
    offs = np.concatenate([[0], np.cumsum(n_ts)]).astype(int)
    nchunks = math.ceil(L / P)
    LP = nchunks * P

    nc = bacc.Bacc("TRN2", target_bir_lowering=False, debug=False)

    xT_d = nc.dram_tensor("xT", [E, L], BF16, kind="ExternalInput")
    wih_d = nc.dram_tensor("wih", [E, 4 * H], BF16, kind="ExternalInput")
    whh_d = nc.dram_tensor("whh", [H, 4 * H], F32R, kind="ExternalInput")
    h0T_d = nc.dram_tensor("h0T", [H, B], F32R, kind="ExternalInput")
    c0_d = nc.dram_tensor("c0", [B, H], F32, kind="ExternalInput")
    id_d = nc.dram_tensor("ident", [P, P], F32R, kind="ExternalInput")
    linT_d = nc.dram_tensor("linT", [H, VS], BF16, kind="ExternalInput")
    bias_d = linb_d = None
    if use_bias:
        bias_d = nc.dram_tensor("bias2", [1, 4 * H], BF16, kind="ExternalInput")
    if use_linb:
        linb_d = nc.dram_tensor("linb", [1, VS], BF16, kind="ExternalInput")
    out_d = nc.dram_tensor("out", [L, VS], F32, kind="ExternalOutput")
    xg_d = hsb_d = None
    if debug_dump:
        xg_d = nc.dram_tensor("xg_dump", [nchunks * P, 4 * 512], BF16, kind="ExternalOutput")
        hsb_d = nc.dram_tensor("hsb_dump", [KH * P, nchunks * P], BF16, kind="ExternalOutput")

    PS = bass.MemorySpace.PSUM

    with tile.TileContext(nc) as tc:
        with (
            tc.tile_pool(name="persist", bufs=1) as pers,
            tc.tile_pool(name="hT", bufs=2) as hTp,
            tc.tile_pool(name="cc", bufs=2) as ccp,
            tc.tile_pool(name="ifo", bufs=2) as ifop,
            tc.tile_pool(name="work", bufs=4) as wkp,
            tc.tile_pool(name="stg", bufs=2) as stgp,
            tc.tile_pool(name="outs", bufs=3) as otp,
            tc.tile_pool(name="gps", bufs=1, space=PS) as gpsp,
            tc.tile_pool(name="tps", bufs=1, space=PS) as tpsp,
            tc.tile_pool(name="pps", bufs=3, space=PS) as ppsp,
        ):
            # ---- small state first ----
            ident = pers.tile([P, P], F32R, tag="ident")
            nc.sync.dma_start(ident[:], id_d[:])
            hT_prev = hTp.tile([P, KH, P], F32R, tag="hT", name="hT0")
            for k in range(KH):
                nc.sync.dma_start(hT_prev[:, k, :], h0T_d[P * k : P * (k + 1), :])
            c_prev = ccp.tile([P, H], F32, tag="c", name="c0t")
            nc.sync.dma_start(c_prev[:], c0_d[:])

            # ---- resident weights & inputs ----
            # x-phase weights (gate-major so j=0 quarters start early)
            wih = [pers.tile([P, 4, 512], BF16, tag=f"wih{k}", name=f"wih{k}") for k in range(KE)]
            for j in range(4):
                for k in range(KE):
                    nc.sync.dma_start(
                        wih[k][:, j, :], wih_d[P * k : P * (k + 1), 512 * j : 512 * (j + 1)]
                    )
            xt = [pers.tile([P, L], BF16, tag=f"xt{k}", name=f"xt{k}") for k in range(KE)]
            for k in range(KE):
                nc.sync.dma_start(xt[k][:], xT_d[P * k : P * (k + 1), :])
            whh = [pers.tile([P, 4, 512], F32R, tag=f"whh{k}", name=f"whh{k}") for k in range(KH)]
            for j in range(4):
                for k in range(KH):
                    nc.sync.dma_start(
                        whh[k][:, j, :], whh_d[P * k : P * (k + 1), 512 * j : 512 * (j + 1)]
                    )
            lt = [pers.tile([P, KH, VC], BF16, tag=f"lt{v}", name=f"lt{v}") for v in range(NV)]
            for v in range(NV):
                for k in range(KH):
                    nc.sync.dma_start(
                        lt[v][:, k, :], linT_d[P * k : P * (k + 1), VC * v : VC * (v + 1)]
                    )
            bias_t = ones_t = linb_t = None
            if use_bias or use_linb:
                ones_t = pers.tile([1, P], BF16, tag="ones")
                nc.vector.memset(ones_t[:], 1.0)
            if use_bias:
                bias_t = pers.tile([1, 4 * H], BF16, tag="bias2")
                nc.sync.dma_start(bias_t[:], bias_d[:])
            if use_linb:
                linb_t = pers.tile([1, VS], BF16, tag="linb")
                nc.sync.dma_start(linb_t[:], linb_d[:])

            # hoisted x-part gates, [P, 4, 512] bf16 per 128-row chunk
            xg = [pers.tile([P, 4, 512], BF16, tag=f"xg{c}", name=f"xg{c}") for c in range(nchunks)]
            # packed transposed hidden-state stash: hsb[:, k, l] = h_l[128k + p]
            hsb = pers.tile([P, KH, LP], BF16, tag="hsb")

            # ---- x-phase: XG[c] = X[chunk c] @ w_ih.T (+ bias), all upfront ----
            # doubles as the PE warmup; copies alternate scalar/vector
            for c in range(nchunks):
                mc = min(P, L - P * c)
                for j in range(4):
                    pp = ppsp.tile([P, 512], F32, tag="pp", name="ppx")
                    for k in range(KE):
                        nc.tensor.matmul(
                            pp[:mc, :], xt[k][:, P * c : P * c + mc], wih[k][:, j, :],
                            start=(k == 0), stop=(k == KE - 1) and not use_bias,
                        )
                    if use_bias:
                        nc.tensor.matmul(
                            pp[:mc, :], ones_t[:1, :mc],
                            bias_t[:1, 512 * j : 512 * (j + 1)],
                            start=False, stop=True,
                        )
                    if (c + j) % 2 == 0:
                        nc.scalar.copy(xg[c][:mc, j, :], pp[:mc, :])
                    else:
                        nc.vector.tensor_copy(xg[c][:mc, j, :], pp[:mc, :])

            # ---- projection work queue (PE filler between steps + tail) ----
            proj_items = []   # (c, v) ready to emit
            proj_ptr = [0]

            def emit_proj(item):
                c, v = item
                mc = min(P, L - P * c)
                pp = ppsp.tile([P, 512], F32, tag="pp", name="ppv")
                for k in range(KH):
                    nc.tensor.matmul(
                        pp[:mc, :VC],
                        hsb[:, k, P * c : P * c + mc],
                        lt[v][:, k, :],
                        start=(k == 0),
                        stop=(k == KH - 1) and not use_linb,
                    )
                if use_linb:
                    nc.tensor.matmul(
                        pp[:mc, :VC], ones_t[:1, :mc],
                        linb_t[:1, VC * v : VC * (v + 1)],
                        start=False, stop=True,
                    )
                ot = otp.tile([P, VC], F32, tag="ot", name="ot")
                if (c + v) % 2 == 0:
                    nc.scalar.copy(ot[:mc, :], pp[:mc, :VC])
                else:
                    nc.vector.tensor_copy(ot[:mc, :], pp[:mc, :VC])
                nc.sync.dma_start(
                    out_d[P * c : P * c + mc, VC * v : VC * (v + 1)], ot[:mc, :]
                )

            def emit_filler(budget):
                while budget > 0 and proj_ptr[0] < len(proj_items):
                    emit_proj(proj_items[proj_ptr[0]])
                    proj_ptr[0] += 1
                    budget -= 1

            # ---- per-step XG staging (partition-aligned view for prefill) ----
            stage = {}

            def emit_stage(t):
                if t >= len(n_ts):
                    return
                n = int(n_ts[t])
                off = int(offs[t])
                c0i, p0 = off // P, off % P
                if p0 == 0:
                    stage[t] = (xg[c0i], n)
                    return
                st = stgp.tile([P, 4, 512], BF16, tag="st", name=f"st{t}")
                a = min(P - p0, n)
                nc.sync.dma_start(st[0:a], xg[c0i][p0 : p0 + a])
                if n > a:
                    nc.sync.dma_start(st[a:n], xg[c0i + 1][0 : n - a])
                stage[t] = (st, n)

            emit_stage(0)
            emit_stage(1)

            # ---- recurrence over packed steps ----
            done_chunks = 0
            for t, n in enumerate(n_ts):
                n = int(n)
                off = int(offs[t])
                n_next = int(n_ts[t + 1]) if t + 1 < len(n_ts) else 0

                # gates: h-part via PSUM matmul, then add the hoisted x-part
                g4 = gpsp.tile([P, 4, 512], F32, tag="g4", name="g4")
                src, _sn = stage.pop(t)
                emit_stage(t + 2)
                for k in range(KH):
                    for j in range(4):
                        nc.tensor.matmul(
                            g4[:n, j, :], hT_prev[:, k, :n], whh[k][:, j, :],
                            start=(k == 0), stop=(k == KH - 1),
                        )
                nc.vector.tensor_add(g4[:n], g4[:n], src[:n])

                # nonlinearities: banks [i,f,o] one fused sigmoid, g tanh
                ifo = ifop.tile([P, 3, 512], F32, tag="ifo", name="ifo")
                nc.scalar.activation(ifo[:n], g4[:n, 0:3, :], AF.Sigmoid)
                g_s = wkp.tile([P, 512], F32, tag="wk", name="gs")
                nc.scalar.activation(g_s[:n, :], g4[:n, 3, :], AF.Tanh)
                t2 = wkp.tile([P, 512], F32, tag="wk", name="t2")
                nc.vector.tensor_mul(t2[:n, :], ifo[:n, 1, :], c_prev[:n, :])
                t1 = wkp.tile([P, 512], F32, tag="wk", name="t1")
                nc.vector.tensor_mul(t1[:n, :], ifo[:n, 0, :], g_s[:n, :])
                c_new = ccp.tile([P, H], F32, tag="c", name="cn")
                nc.vector.tensor_add(c_new[:n, :], t1[:n, :], t2[:n, :])
                tct = wkp.tile([P, 512], F32, tag="wk", name="tct")
                nc.scalar.activation(tct[:n, :], c_new[:n, :], AF.Tanh)
                h_sb = wkp.tile([P, 512], F32R, tag="wkh", bufs=2, name="hsbt")
                nc.vector.tensor_mul(h_sb[:n, :], ifo[:n, 2, :], tct[:n, :])

                # PE filler while the act/vector chain runs
                emit_filler(F1)

                # transpose h back to [H, n] (PE, one PSUM bank)
                psT = tpsp.tile([P, KH, P], F32R, tag="tp", name="psT")
                for k in range(KH):
                    nc.tensor.transpose(
                        psT[:, k, :n], h_sb[:n, P * k : P * (k + 1)], ident[:n, :n]
                    )

                # hT state for the next step (critical path: emit first)
                if n_next > 0:
                    hT_new = hTp.tile([P, KH, P], F32R, tag="hT", name="hTn")
                    nc.vector.tensor_copy(hT_new[:, :, :n_next], psT[:, :, :n_next])
                    hT_prev = hT_new
                c_prev = c_new

                # stash into the bf16 [H, L] projection input
                nc.vector.tensor_copy(hsb[:, :, off : off + n], psT[:, :, :n])

                emit_filler(F2)

                # chunks fully stashed -> their projection items become ready
                while (done_chunks + 1) * P <= int(offs[t + 1]) or (
                    t == len(n_ts) - 1 and done_chunks < nchunks
                ):
                    for v in range(NV):
                        proj_items.append((done_chunks, v))
                    done_chunks += 1

            # ---- projection: remaining work ----
            while proj_ptr[0] < len(proj_items):
                emit_proj(proj_items[proj_ptr[0]])
                proj_ptr[0] += 1

            if debug_dump:
                for c in range(nchunks):
                    nc.sync.dma_start(xg_d[P * c : P * (c + 1), :], xg[c][:])
                for k in range(KH):
                    nc.sync.dma_start(hsb_d[P * k : P * (k + 1), :], hsb[:, k, :])

    nc.compile()
    return nc


_prog_cache = {}


def _get_program(n_ts, use_bias, use_linb):
    key = (tuple(int(x) for x in n_ts), bool(use_bias), bool(use_linb))
    if key not in _prog_cache:
        _prog_cache[key] = build_program(n_ts, use_bias, use_linb)
    return _prog_cache[key]


def kernel(
    features,
    captions,
    lengths,
    h0,
    c0,
    embed_w,
    w_ih,
    w_hh,
    b_ih,
    b_hh,
    lin_w,
    lin_b,
    maxlen,
    _trace=False,
):
    bf16 = mybir.dt.np(BF16)
    features = np.asarray(features, np.float32)
    captions = np.asarray(captions)
    lengths = np.asarray(lengths)
    h0 = np.asarray(h0, np.float32)
    c0 = np.asarray(c0, np.float32)
    embed_w = np.asarray(embed_w, np.float32)
    w_ih = np.asarray(w_ih, np.float32)
    w_hh = np.asarray(w_hh, np.float32)
    b_ih = np.asarray(b_ih, np.float32)
    b_hh = np.asarray(b_hh, np.float32)
    lin_w = np.asarray(lin_w, np.float32)
    lin_b = np.asarray(lin_b, np.float32)
    maxlen = int(maxlen)
    batch = captions.shape[0]

    # Sort rows by descending length (stable). pack_padded_sequence requires
    # descending lengths, so perm is normally the identity; the permutation
    # fallback keeps us correct on arbitrary length order.
    ln = lengths.astype(np.int64)
    perm = np.argsort(-ln, kind="stable")
    identity_perm = bool(np.all(perm == np.arange(batch)))
    lns = ln[perm]

    n_ts = []
    for t in range(maxlen):
        n = int((lns > t).sum())
        if n == 0:
            break
        n_ts.append(n)
    L = int(sum(n_ts))
    offs = np.concatenate([[0], np.cumsum(n_ts)]).astype(int)

    # host prep: packed transposed input sequence xT [E, L]
    xs = np.empty((L, E), np.float32)
    for t, n in enumerate(n_ts):
        sel = perm[:n]
        if t == 0:
            xs[offs[t] : offs[t] + n] = features[sel]
        else:
            xs[offs[t] : offs[t] + n] = embed_w[captions[sel, t - 1]]
    xT = np.ascontiguousarray(xs.T).astype(bf16)

    # gate blocks reordered [i,f,g,o] -> [i,f,o,g]
    def gate_permute_cols(wT):
        blocks = [wT[:, 512 * j : 512 * (j + 1)] for j in GPERM]
        return np.ascontiguousarray(np.concatenate(blocks, axis=1))

    wihT = gate_permute_cols(w_ih.T.astype(np.float32)).astype(bf16)
    whhT = gate_permute_cols(np.ascontiguousarray(w_hh.T))
    h0T = np.ascontiguousarray(h0[perm].T)
    c0p = np.ascontiguousarray(c0[perm])
    linT = np.ascontiguousarray(lin_w.T).astype(bf16)
    ident = np.eye(P, dtype=np.float32)
    bias2 = (b_ih + b_hh).astype(np.float32)
    bias2 = np.concatenate([bias2[512 * j : 512 * (j + 1)] for j in GPERM])
    use_bias = bool(np.any(bias2))
    use_linb = bool(np.any(lin_b))

    nc = _get_program(n_ts, use_bias, use_linb)

    in_maps = []
    for s in range(NCORES):
        m = {
            "xT": xT,
            "wih": wihT,
            "whh": whhT,
            "h0T": h0T,
            "c0": c0p,
            "ident": ident,
            "linT": np.ascontiguousarray(linT[:, VS * s : VS * (s + 1)]),
        }
        if use_bias:
            m["bias2"] = bias2.reshape(1, 4 * H).astype(bf16)
        if use_linb:
            m["linb"] = np.ascontiguousarray(
                lin_b[VS * s : VS * (s + 1)].reshape(1, VS)
            ).astype(bf16)
        in_maps.append(m)

    res = run_bass_kernel_spmd(
        nc, in_maps, core_ids=list(range(NCORES)), trace=_trace
    )
    out = np.concatenate([np.asarray(r["out"]) for r in res.results], axis=1)

    if not identity_perm:
        # map packed rows computed in sorted order back to original order
        src = np.empty(L, np.int64)
        pos = 0
        inv_pos = {}
        for t, n in enumerate(n_ts):
            for j in range(n):
                inv_pos[(t, int(perm[j]))] = offs[t] + j
        for t in range(maxlen):
            for i in np.nonzero(ln > t)[0]:
                src[pos] = inv_pos[(t, int(i))]
                pos += 1
        out = out[src]

    if _trace:
        return out, res
    return out


# revision 15
# speedup vs baseline: 1.3274x; 1.3274x over previous
"""Trainium2 Bass kernel: teacher-forced LSTM decoder + packed vocab projection.

Model (B=128, T=20, E=H=512, V=32000):
  x = [features, embed(captions[:, :T-1])]            # [B, T, E]
  (h, c) LSTM-scan over T steps (PyTorch gate order i,f,g,o)
  logits = hs @ lin_w.T + lin_b                       # [T, B, V]
  out = logits packed time-major, keeping rows with length > t  # [sum(len), V]

Strategy (8 NeuronCores, vocab-parallel):
  - Core s owns lin_w columns [s*4000, (s+1)*4000); the LSTM recurrence is
    replicated on every core.
  - x-part gates are HOISTED out of the recurrence: XG[L, 4H] = X @ w_ih.T is
    computed in one chunk-batched bf16 pass over the packed rows (13 chunks of
    128 instead of 20 per-step groups -> fewer weight-column streams), and
    doubles as the PE p-state warmup.
  - Per step t, the 4 gate PSUM banks are PREFILLED with XG rows (one scalar
    copy), then only the h-part accumulates via matmul (start=False):
      gates[n,4H] = prefill(XG) + h_{t-1} @ w_hh.T     (f32r, 4K x 4 banks)
    Gate columns are host-permuted to [i,f,o,g] so one fused sigmoid covers
    banks 0..2 and one tanh covers bank 3.
  - c update in fp32 on DVE, h transposed back to [H, n] via PE transpose,
    stash into a bf16 [H, L] tile feeding the projection.
  - Projection: per (row-chunk, 500-wide vocab slice), bf16 matmul vs resident
    bf16 linT; items are interleaved between recurrence steps as PE filler.
  - Host gathers the 8 core outputs and concatenates along vocab.
"""

import math

import numpy as np

import concourse.bacc as bacc
import concourse.bass as bass
import concourse.mybir as mybir
import concourse.tile as tile
from concourse.bass_utils import run_bass_kernel_spmd

B, T, E, H, V = 128, 20, 512, 512, 32000
NCORES = 8
VS = V // NCORES      # per-core vocab shard (4000)
NV = 8                # vocab sub-chunks per core
VC = VS // NV         # 500 columns per projection matmul (>=256 keeps rate 1c/r)
KE = E // 128         # 4 contraction chunks over E
KH = H // 128         # 4 contraction chunks over H
P = 128

F32 = mybir.dt.float32
F32R = mybir.dt.float32r
BF16 = mybir.dt.bfloat16
AF = mybir.ActivationFunctionType

# gate blocks reordered host-side from PyTorch [i,f,g,o] to [i,f,o,g] so the
# three sigmoids are contiguous PSUM banks 0..2 and tanh is bank 3
GPERM = (0, 1, 3, 2)

# PE filler sizing: projection items emitted between a step's h-matmuls and
# its transpose (F1) and after the transpose (F2)
F1 = 4
F2 = 1


def build_program(n_ts, use_bias, use_linb, debug_dump=False):
    """Build the single-core Bass/Tile program (same program on all 8 cores).

    n_ts: per-step active-row counts (descending, all > 0), len(n_ts) <= T.
    """
    L = int(sum(n_ts))
    offs = np.concatenate([[0], np.cumsum(n_ts)]).astype(int)
    nchunks = math.ceil(L / P)
    LP = nchunks * P

    nc = bacc.Bacc("TRN2", target_bir_lowering=False, debug=False)

    xT_d = nc.dram_tensor("xT", [E, L], BF16, kind="ExternalInput")
    wih_d = nc.dram_tensor("wih", [E, 4 * H], BF16, kind="ExternalInput")
    whh_d = nc.dram_tensor("whh", [H, 4 * H], F32R, kind="ExternalInput")
    h0T_d = nc.dram_tensor("h0T", [H, B], F32R, kind="ExternalInput")
    c0_d = nc.dram_tensor("c0", [B, H], F32, kind="ExternalInput")
    id_d = nc.dram_tensor("ident", [P, P], F32R, kind="ExternalInput")
    idb_d = nc.dram_tensor("identb", [P, P], BF16, kind="ExternalInput")
    linT_d = nc.dram_tensor("linT", [H, VS], BF16, kind="ExternalInput")
    bias_d = linb_d = None
    if use_bias:
        bias_d = nc.dram_tensor("bias2", [1, 4 * H], BF16, kind="ExternalInput")
    if use_linb:
        linb_d = nc.dram_tensor("linb", [1, VS], BF16, kind="ExternalInput")
    out_d = nc.dram_tensor("out", [L, VS], F32, kind="ExternalOutput")
    xg_d = hsb_d = None
    if debug_dump:
        xg_d = nc.dram_tensor("xg_dump", [nchunks * P, 4 * 512], BF16, kind="ExternalOutput")
        hsb_d = nc.dram_tensor("hsb_dump", [KH * P, nchunks * P], BF16, kind="ExternalOutput")

    PS = bass.MemorySpace.PSUM

    with tile.TileContext(nc) as tc:
        with (
            tc.tile_pool(name="persist", bufs=1) as pers,
            tc.tile_pool(name="hT", bufs=2) as hTp,
            tc.tile_pool(name="cc", bufs=2) as ccp,
            tc.tile_pool(name="ifo", bufs=2) as ifop,
            tc.tile_pool(name="work", bufs=4) as wkp,
            tc.tile_pool(name="stg", bufs=2) as stgp,
            tc.tile_pool(name="outs", bufs=3) as otp,
            tc.tile_pool(name="gps", bufs=1, space=PS) as gpsp,
            tc.tile_pool(name="tps", bufs=1, space=PS) as tpsp,
            tc.tile_pool(name="pps", bufs=3, space=PS) as ppsp,
        ):
            # ---- small state first ----
            ident = pers.tile([P, P], F32R, tag="ident")
            nc.sync.dma_start(ident[:], id_d[:])
            identb = pers.tile([P, P], BF16, tag="identb")
            nc.sync.dma_start(identb[:], idb_d[:])
            hT_prev = hTp.tile([P, KH, P], F32R, tag="hT", name="hT0")
            for k in range(KH):
                nc.sync.dma_start(hT_prev[:, k, :], h0T_d[P * k : P * (k + 1), :])
            c_prev = ccp.tile([P, H], F32, tag="c", name="c0t")
            nc.sync.dma_start(c_prev[:], c0_d[:])

            # ---- resident weights & inputs ----
            # x-phase weights (gate-major so j=0 quarters start early)
            wih = [pers.tile([P, 4, 512], BF16, tag=f"wih{k}", name=f"wih{k}") for k in range(KE)]
            for j in range(4):
                for k in range(KE):
                    nc.sync.dma_start(
                        wih[k][:, j, :], wih_d[P * k : P * (k + 1), 512 * j : 512 * (j + 1)]
                    )
            xt = [pers.tile([P, L], BF16, tag=f"xt{k}", name=f"xt{k}") for k in range(KE)]
            for k in range(KE):
                nc.sync.dma_start(xt[k][:], xT_d[P * k : P * (k + 1), :])
            whh = [pers.tile([P, 4, 512], F32R, tag=f"whh{k}", name=f"whh{k}") for k in range(KH)]
            for j in range(4):
                for k in range(KH):
                    nc.sync.dma_start(
                        whh[k][:, j, :], whh_d[P * k : P * (k + 1), 512 * j : 512 * (j + 1)]
                    )
            lt = [pers.tile([P, KH, VC], BF16, tag=f"lt{v}", name=f"lt{v}") for v in range(NV)]
            for v in range(NV):
                for k in range(KH):
                    nc.sync.dma_start(
                        lt[v][:, k, :], linT_d[P * k : P * (k + 1), VC * v : VC * (v + 1)]
                    )
            bias_t = ones_t = linb_t = None
            if use_bias or use_linb:
                ones_t = pers.tile([1, P], BF16, tag="ones")
                nc.vector.memset(ones_t[:], 1.0)
            if use_bias:
                bias_t = pers.tile([1, 4 * H], BF16, tag="bias2")
                nc.sync.dma_start(bias_t[:], bias_d[:])
            if use_linb:
                linb_t = pers.tile([1, VS], BF16, tag="linb")
                nc.sync.dma_start(linb_t[:], linb_d[:])

            # hoisted x-part gates, [P, 4, 512] bf16 per 128-row chunk
            xg = [pers.tile([P, 4, 512], BF16, tag=f"xg{c}", name=f"xg{c}") for c in range(nchunks)]
            # packed transposed hidden-state stash: hsb[:, k, l] = h_l[128k + p]
            hsb = pers.tile([P, KH, LP], BF16, tag="hsb")

            # ---- x-phase: XG[c] = X[chunk c] @ w_ih.T (+ bias), all upfront ----
            # doubles as the PE warmup; copies alternate scalar/vector
            for c in range(nchunks):
                mc = min(P, L - P * c)
                for j in range(4):
                    pp = ppsp.tile([P, 512], F32, tag="pp", name="ppx")
                    for k in range(KE):
                        nc.tensor.matmul(
                            pp[:mc, :], xt[k][:, P * c : P * c + mc], wih[k][:, j, :],
                            start=(k == 0), stop=(k == KE - 1) and not use_bias,
                        )
                    if use_bias:
                        nc.tensor.matmul(
                            pp[:mc, :], ones_t[:1, :mc],
                            bias_t[:1, 512 * j : 512 * (j + 1)],
                            start=False, stop=True,
                        )
                    if (c + j) % 2 == 0:
                        nc.scalar.copy(xg[c][:mc, j, :], pp[:mc, :])
                    else:
                        nc.vector.tensor_copy(xg[c][:mc, j, :], pp[:mc, :])

            # ---- projection work queue (PE filler between steps + tail) ----
            proj_items = []   # (c, v) ready to emit
            proj_ptr = [0]

            def emit_proj(item):
                c, v = item
                mc = min(P, L - P * c)
                pp = ppsp.tile([P, 512], F32, tag="pp", name="ppv")
                for k in range(KH):
                    nc.tensor.matmul(
                        pp[:mc, :VC],
                        hsb[:, k, P * c : P * c + mc],
                        lt[v][:, k, :],
                        start=(k == 0),
                        stop=(k == KH - 1) and not use_linb,
                    )
                if use_linb:
                    nc.tensor.matmul(
                        pp[:mc, :VC], ones_t[:1, :mc],
                        linb_t[:1, VC * v : VC * (v + 1)],
                        start=False, stop=True,
                    )
                ot = otp.tile([P, VC], F32, tag="ot", name="ot")
                if (c + v) % 2 == 0:
                    nc.scalar.copy(ot[:mc, :], pp[:mc, :VC])
                else:
                    nc.vector.tensor_copy(ot[:mc, :], pp[:mc, :VC])
                nc.sync.dma_start(
                    out_d[P * c : P * c + mc, VC * v : VC * (v + 1)], ot[:mc, :]
                )

            def emit_filler(budget):
                while budget > 0 and proj_ptr[0] < len(proj_items):
                    emit_proj(proj_items[proj_ptr[0]])
                    proj_ptr[0] += 1
                    budget -= 1

            # ---- per-step XG staging (partition-aligned view for prefill) ----
            stage = {}

            def emit_stage(t):
                if t >= len(n_ts):
                    return
                n = int(n_ts[t])
                off = int(offs[t])
                c0i, p0 = off // P, off % P
                if p0 == 0:
                    stage[t] = (xg[c0i], n)
                    return
                st = stgp.tile([P, 4, 512], BF16, tag="st", name=f"st{t}")
                a = min(P - p0, n)
                nc.sync.dma_start(st[0:a], xg[c0i][p0 : p0 + a])
                if n > a:
                    nc.sync.dma_start(st[a:n], xg[c0i + 1][0 : n - a])
                stage[t] = (st, n)

            emit_stage(0)
            emit_stage(1)

            # ---- recurrence over packed steps ----
            done_chunks = 0
            for t, n in enumerate(n_ts):
                n = int(n)
                off = int(offs[t])
                n_next = int(n_ts[t + 1]) if t + 1 < len(n_ts) else 0

                # gates: identity-select matmul seeds each bank with the
                # hoisted x-part (start=True), then the h-part accumulates
                g4 = gpsp.tile([P, 4, 512], F32, tag="g4", name="g4")
                src, _sn = stage.pop(t)
                emit_stage(t + 2)
                for j in range(4):
                    nc.tensor.matmul(
                        g4[:n, j, :], identb[:n, :n], src[:n, j, :],
                        start=True, stop=False,
                    )
                for j in range(4):
                    for k in range(KH):
                        nc.tensor.matmul(
                            g4[:n, j, :], hT_prev[:, k, :n], whh[k][:, j, :],
                            start=False, stop=(k == KH - 1),
                        )

                # nonlinearities: banks [i,f,o] one fused sigmoid, g tanh
                ifo = ifop.tile([P, 3, 512], F32, tag="ifo", name="ifo")
                nc.scalar.activation(ifo[:n], g4[:n, 0:3, :], AF.Sigmoid)
                g_s = wkp.tile([P, 512], F32, tag="wk", name="gs")
                nc.scalar.activation(g_s[:n, :], g4[:n, 3, :], AF.Tanh)
                t2 = wkp.tile([P, 512], F32, tag="wk", name="t2")
                nc.vector.tensor_mul(t2[:n, :], ifo[:n, 1, :], c_prev[:n, :])
                t1 = wkp.tile([P, 512], F32, tag="wk", name="t1")
                nc.vector.tensor_mul(t1[:n, :], ifo[:n, 0, :], g_s[:n, :])
                c_new = ccp.tile([P, H], F32, tag="c", name="cn")
                nc.vector.tensor_add(c_new[:n, :], t1[:n, :], t2[:n, :])
                tct = wkp.tile([P, 512], F32, tag="wk", name="tct")
                nc.scalar.activation(tct[:n, :], c_new[:n, :], AF.Tanh)
                h_sb = wkp.tile([P, 512], F32R, tag="wkh", bufs=2, name="hsbt")
                nc.vector.tensor_mul(h_sb[:n, :], ifo[:n, 2, :], tct[:n, :])

                # PE filler while the act/vector chain runs
                emit_filler(F1)

                # transpose h back to [H, n] (PE, one PSUM bank)
                psT = tpsp.tile([P, KH, P], F32R, tag="tp", name="psT")
                for k in range(KH):
                    nc.tensor.transpose(
                        psT[:, k, :n], h_sb[:n, P * k : P * (k + 1)], ident[:n, :n]
                    )

                # hT state for the next step (critical path: emit first)
                if n_next > 0:
                    hT_new = hTp.tile([P, KH, P], F32R, tag="hT", name="hTn")
                    nc.vector.tensor_copy(hT_new[:, :, :n_next], psT[:, :, :n_next])
                    hT_prev = hT_new
                c_prev = c_new

                # stash into the bf16 [H, L] projection input (scalar engine:
                # DVE keeps the c/h chain, ACT has slack and fast PSUM reads)
                nc.scalar.copy(hsb[:, :, off : off + n], psT[:, :, :n])

                emit_filler(F2)

                # chunks fully stashed -> their projection items become ready
                while (done_chunks + 1) * P <= int(offs[t + 1]) or (
                    t == len(n_ts) - 1 and done_chunks < nchunks
                ):
                    for v in range(NV):
                        proj_items.append((done_chunks, v))
                    done_chunks += 1

            # ---- projection: remaining work ----
            while proj_ptr[0] < len(proj_items):
                emit_proj(proj_items[proj_ptr[0]])
                proj_ptr[0] += 1

            if debug_dump:
                for c in range(nchunks):
                    nc.sync.dma_start(xg_d[P * c : P * (c + 1), :], xg[c][:])
                for k in range(KH):
                    nc.sync.dma_start(hsb_d[P * k : P * (k + 1), :], hsb[:, k, :])

    nc.compile()
    return nc


_prog_cache = {}


def _get_program(n_ts, use_bias, use_linb):
    key = (tuple(int(x) for x in n_ts), bool(use_bias), bool(use_linb))
    if key not in _prog_cache:
        _prog_cache[key] = build_program(n_ts, use_bias, use_linb)
    return _prog_cache[key]


def kernel(
    features,
    captions,
    lengths,
    h0,
    c0,
    embed_w,
    w_ih,
    w_hh,
    b_ih,
    b_hh,
    lin_w,
    lin_b,
    maxlen,
    _trace=False,
):
    bf16 = mybir.dt.np(BF16)
    features = np.asarray(features, np.float32)
    captions = np.asarray(captions)
    lengths = np.asarray(lengths)
    h0 = np.asarray(h0, np.float32)
    c0 = np.asarray(c0, np.float32)
    embed_w = np.asarray(embed_w, np.float32)
    w_ih = np.asarray(w_ih, np.float32)
    w_hh = np.asarray(w_hh, np.float32)
    b_ih = np.asarray(b_ih, np.float32)
    b_hh = np.asarray(b_hh, np.float32)
    lin_w = np.asarray(lin_w, np.float32)
    lin_b = np.asarray(lin_b, np.float32)
    maxlen = int(maxlen)
    batch = captions.shape[0]

    # Sort rows by descending length (stable). pack_padded_sequence requires
    # descending lengths, so perm is normally the identity; the permutation
    # fallback keeps us correct on arbitrary length order.
    ln = lengths.astype(np.int64)
    perm = np.argsort(-ln, kind="stable")
    identity_perm = bool(np.all(perm == np.arange(batch)))
    lns = ln[perm]

    n_ts = []
    for t in range(maxlen):
        n = int((lns > t).sum())
        if n == 0:
            break
        n_ts.append(n)
    L = int(sum(n_ts))
    offs = np.concatenate([[0], np.cumsum(n_ts)]).astype(int)

    # host prep: packed transposed input sequence xT [E, L]
    xs = np.empty((L, E), np.float32)
    for t, n in enumerate(n_ts):
        sel = perm[:n]
        if t == 0:
            xs[offs[t] : offs[t] + n] = features[sel]
        else:
            xs[offs[t] : offs[t] + n] = embed_w[captions[sel, t - 1]]
    xT = np.ascontiguousarray(xs.T).astype(bf16)

    # gate blocks reordered [i,f,g,o] -> [i,f,o,g]
    def gate_permute_cols(wT):
        blocks = [wT[:, 512 * j : 512 * (j + 1)] for j in GPERM]
        return np.ascontiguousarray(np.concatenate(blocks, axis=1))

    wihT = gate_permute_cols(w_ih.T.astype(np.float32)).astype(bf16)
    whhT = gate_permute_cols(np.ascontiguousarray(w_hh.T))
    h0T = np.ascontiguousarray(h0[perm].T)
    c0p = np.ascontiguousarray(c0[perm])
    linT = np.ascontiguousarray(lin_w.T).astype(bf16)
    ident = np.eye(P, dtype=np.float32)
    bias2 = (b_ih + b_hh).astype(np.float32)
    bias2 = np.concatenate([bias2[512 * j : 512 * (j + 1)] for j in GPERM])
    use_bias = bool(np.any(bias2))
    use_linb = bool(np.any(lin_b))

    nc = _get_program(n_ts, use_bias, use_linb)

    in_maps = []
    for s in range(NCORES):
        m = {
            "xT": xT,
            "wih": wihT,
            "whh": whhT,
            "h0T": h0T,
            "c0": c0p,
            "ident": ident,
            "identb": ident.astype(bf16),
            "linT": np.ascontiguousarray(linT[:, VS * s : VS * (s + 1)]),
        }
        if use_bias:
            m["bias2"] = bias2.reshape(1, 4 * H).astype(bf16)
        if use_linb:
            m["linb"] = np.ascontiguousarray(
                lin_b[VS * s : VS * (s + 1)].reshape(1, VS)
            ).astype(bf16)
        in_maps.append(m)

    res = run_bass_kernel_spmd(
        nc, in_maps, core_ids=list(range(NCORES)), trace=_trace
    )
    out = np.concatenate([np.asarray(r["out"]) for r in res.results], axis=1)

    if not identity_perm:
        # map packed rows computed in sorted order back to original order
        src = np.empty(L, np.int64)
        pos = 0
        inv_pos = {}
        for t, n in enumerate(n_ts):
            for j in range(n):
                inv_pos[(t, int(perm[j]))] = offs[t] + j
        for t in range(maxlen):
            for i in np.nonzero(ln > t)[0]:
                src[pos] = inv_pos[(t, int(i))]
                pos += 1
        out = out[src]

    if _trace:
        return out, res
    return out
